# revision 17
# baseline (speedup 1.0000x reference)
"""AdaptiveStdPool2d kernel for Trainium2 (8 NeuronCores, data-parallel).

Input  x: [32, 64, 512, 80] f32
Output:   [32, 64, 8, 10] f32  (mean/std interleaved along height)

Math: per (b, c), split H=512 into 4 windows of 128 and W=80 into 10
windows of 8; out[b,c,2*oh,ow] = mean of 128x8 window, out[b,c,2*oh+1,ow]
= sqrt(biased_var + 1e-14).

Strategy (per core, shard = 4 batches -> 256 (b,c) rows):
- partition dim = (b,c) row (2 tiles of 128), free dim = flattened (h,w).
- per (bc-tile, oh): one contiguous slab DMA [128, 10240] on the Sync
  HWDGE ring (FIFO -> continuous HBM stream at ~424 GB/s).  The LAST
  slab is DMA'd as rows 0..95 + 96..127 so only a quarter of its work
  trails the final bytes (shorter drain tail).
- output DMAs stay OFF the hot Sync FIFO: tile 0's result goes out on
  the GpSimd SWDGE ring (a result-dependent wait on the Sync ring would
  head-of-line-block input descgen mid-stream; the Scalar HWDGE ring
  measurably perturbs SDMA engine 15 - do not use it); the final
  result uses the Sync ring, which is drained by then and ~1.5us
  faster end-to-end than SWDGE.
- window stats split across engines on every slab: 7 of 10 windows on
  DVE (2x BNStats 512-elem groups + BNStatsAggregate straight into the
  interleaved output layout), 3 on ACT (Square/Copy with scale +
  accum_out giving E[x^2] and mean directly), 8/2 on the last slab.
  Per-slab work on each engine depends ONLY on that slab's DMA - no
  per-slab cross-engine sync, so one straggling DMA engine can't
  cascade.
- finishing (var = E[x^2]-mean^2 for ACT windows, sqrt(var+eps) for
  all) is batched once per bc-tile: a few tiny DVE ops + 1 ACT sqrt.
"""

import os
import numpy as np

B, C, H, W = 32, 64, 512, 80
OUT_H2, OUT_W = 4, 10
WH, WW = H // OUT_H2, W // OUT_W  # 128, 8
EPS = 1e-14
NWIN = WH * WW                   # 1024 elements per window

N_CORES = 8
B_SH = B // N_CORES          # 4 batches per core
BC = B_SH * C                # 256 rows per core
HW = H * W                   # 40960
SLAB = WH * W                # 10240 elements per (oh) slab
HALF = SLAB // 2             # 5120 (rows 0..63 of a slab)
OUT_FREE = 2 * OUT_H2 * OUT_W  # 80 output elements per (b,c)

_CACHE = {}
LAST_RESULTS = None


def _build():
    import concourse.bacc as bacc
    import concourse.tile as tile
    from concourse import mybir

    nc = bacc.Bacc("TRN2", target_bir_lowering=False, debug=False)

    x = nc.dram_tensor("x", [BC, HW], mybir.dt.float32, kind="ExternalInput")
    out = nc.dram_tensor("out", [BC, OUT_FREE], mybir.dt.float32,
                         kind="ExternalOutput")

    P = 128
    NT = BC // P  # 2 bc-tiles
    INV_N = 1.0 / NWIN
    INV_SQRTN = 1.0 / 32.0   # 1/sqrt(1024): Square(x/32) accums to E[x^2]

    def bn_stats_raw(in_ap, out_ap):
        # Raw InstBNStats: HW computes one 6-stat group over the whole
        # (multi-dim) input AP; bass's wrapper would reject this shape.
        nc.vector.add_instruction(
            mybir.InstBNStats(
                name=nc.get_next_instruction_name(),
                ins=[nc.vector.lower_ap(in_ap)],
                outs=[nc.vector.lower_ap(out_ap)],
            )
        )

    with tile.TileContext(nc) as tc:
        with (
            tc.tile_pool(name="slabs", bufs=4) as slabs,
            tc.tile_pool(name="scratch", bufs=2) as scratch_pool,
            tc.tile_pool(name="small", bufs=4) as small,
            tc.tile_pool(name="res", bufs=2) as res_pool,
            tc.tile_pool(name="singles", bufs=1) as singles,
        ):
            eps_t = singles.tile([P, 1], mybir.dt.float32)
            nc.vector.memset(eps_t, EPS)

            for t in range(NT):
                res = res_pool.tile([P, OUT_H2, 2, OUT_W], mybir.dt.float32)
                # E[x^2] for the ACT windows, kept per tile until the
                # batched finishing pass.
                ex2 = small.tile([P, OUT_H2, OUT_W], mybir.dt.float32)
                for oh in range(OUT_H2):
                    last = (t == NT - 1) and (oh == OUT_H2 - 1)
                    n_act = 2 if last else 3
                    n_dve = OUT_W - n_act
                    slab = slabs.tile([P, SLAB], mybir.dt.float32)
                    c0 = oh * SLAB
                    slab_v = slab.rearrange("p (r ow w) -> p ow r w",
                                            ow=OUT_W, w=WW)
                    stats = small.tile([P, OUT_W, 2, 6], mybir.dt.float32)
                    if not last:
                        nc.sync.dma_start(
                            out=slab[:],
                            in_=x[t * P:(t + 1) * P, c0:c0 + SLAB],
                        )
                        # DVE windows: 2x 512-elem column-split groups
                        for ow in range(n_dve):
                            for k in range(2):
                                bn_stats_raw(
                                    slab_v[:, ow, :, k * 4:(k + 1) * 4],
                                    stats[:, ow, k, :],
                                )
                            nc.vector.bn_aggr(out=res[:, oh, :, ow],
                                              in_=stats[:, ow, :, :])
                        # ACT windows: E[x^2] and mean via scaled accum
                        for ow in range(n_dve, OUT_W):
                            sq_scr = scratch_pool.tile([P, WH, WW],
                                                       mybir.dt.float32)
                            nc.scalar.activation(
                                out=sq_scr[:],
                                in_=slab_v[:, ow],
                                func=mybir.ActivationFunctionType.Square,
                                scale=INV_SQRTN,
                                accum_out=ex2[:, oh, ow:ow + 1],
                            )
                            cp_scr = scratch_pool.tile([P, WH, WW],
                                                       mybir.dt.float32)
                            nc.scalar.activation(
                                out=cp_scr[:],
                                in_=slab_v[:, ow],
                                func=mybir.ActivationFunctionType.Copy,
                                scale=INV_N,
                                accum_out=res[:, oh, 0, ow:ow + 1],
                            )
                    else:
                        # Last slab: DMA as rows 0..95 + rows 96..127 so
                        # only a quarter of the work trails the final
                        # chunk.  DVE: 3 row-group BNStats per window
                        # (<=512 elems each); ACT: partial accums per
                        # chunk, summed on DVE.
                        RA = WH * 3 // 4  # 96
                        CA = RA * W       # 7680
                        nc.sync.dma_start(
                            out=slab[:, :CA],
                            in_=x[t * P:(t + 1) * P, c0:c0 + CA],
                        )
                        nc.sync.dma_start(
                            out=slab[:, CA:],
                            in_=x[t * P:(t + 1) * P, c0 + CA:c0 + SLAB],
                        )
                        stats3 = small.tile([P, OUT_W, 3, 6],
                                            mybir.dt.float32)
                        # chunk A work (overlaps chunk-B DMA)
                        for ow in range(n_dve):
                            bn_stats_raw(slab_v[:, ow, 0:WH // 2, :],
                                         stats3[:, ow, 0, :])
                            bn_stats_raw(slab_v[:, ow, WH // 2:RA, :],
                                         stats3[:, ow, 1, :])
                        sm_p = small.tile([P, n_act, 2], mybir.dt.float32)
                        ex_p = small.tile([P, n_act, 2], mybir.dt.float32)
                        for j, ow in enumerate(range(n_dve, OUT_W)):
                            sq_scr = scratch_pool.tile([P, WH, WW],
                                                       mybir.dt.float32)
                            nc.scalar.activation(
                                out=sq_scr[:, :RA],
                                in_=slab_v[:, ow, :RA],
                                func=mybir.ActivationFunctionType.Square,
                                scale=INV_SQRTN,
                                accum_out=ex_p[:, j, 0:1],
                            )
                            cp_scr = scratch_pool.tile([P, WH, WW],
                                                       mybir.dt.float32)
                            nc.scalar.activation(
                                out=cp_scr[:, :RA],
                                in_=slab_v[:, ow, :RA],
                                func=mybir.ActivationFunctionType.Copy,
                                scale=INV_N,
                                accum_out=sm_p[:, j, 0:1],
                            )
                        # chunk B work (the short tail)
                        for ow in range(n_dve):
                            bn_stats_raw(slab_v[:, ow, RA:, :],
                                         stats3[:, ow, 2, :])
                            nc.vector.bn_aggr(out=res[:, oh, :, ow],
                                              in_=stats3[:, ow, :, :])
                        for j, ow in enumerate(range(n_dve, OUT_W)):
                            sq_scr = scratch_pool.tile([P, WH, WW],
                                                       mybir.dt.float32)
                            nc.scalar.activation(
                                out=sq_scr[:, RA:],
                                in_=slab_v[:, ow, RA:],
                                func=mybir.ActivationFunctionType.Square,
                                scale=INV_SQRTN,
                                accum_out=ex_p[:, j, 1:2],
                            )
                            cp_scr = scratch_pool.tile([P, WH, WW],
                                                       mybir.dt.float32)
                            nc.scalar.activation(
                                out=cp_scr[:, RA:],
                                in_=slab_v[:, ow, RA:],
                                func=mybir.ActivationFunctionType.Copy,
                                scale=INV_N,
                                accum_out=sm_p[:, j, 1:2],
                            )
                        # combine chunk partials
                        nc.vector.tensor_add(res[:, oh, 0, n_dve:],
                                             sm_p[:, :, 0], sm_p[:, :, 1])
                        nc.vector.tensor_add(ex2[:, oh, n_dve:],
                                             ex_p[:, :, 0], ex_p[:, :, 1])
                # Batched finishing for the whole tile.
                # var for ACT windows: E[x^2] - mean^2.  The last slab
                # of the last tile ran only ow 8:10 on ACT (ow 7 came
                # from bn_aggr), so its range is ragged.
                m2 = small.tile([P, OUT_H2, OUT_W], mybir.dt.float32)
                if t == NT - 1:
                    regions = [(slice(0, OUT_H2 - 1), slice(7, OUT_W)),
                               (OUT_H2 - 1, slice(8, OUT_W))]
                else:
                    regions = [(slice(0, OUT_H2), slice(7, OUT_W))]
                for rh, rw in regions:
                    nc.vector.tensor_mul(m2[:, rh, rw],
                                         res[:, rh, 0, rw],
                                         res[:, rh, 0, rw])
                    nc.vector.scalar_tensor_tensor(
                        out=res[:, rh, 1, rw],
                        in0=ex2[:, rh, rw],
                        scalar=1.0,
                        in1=m2[:, rh, rw],
                        op0=mybir.AluOpType.mult,
                        op1=mybir.AluOpType.subtract,
                    )
                # std = sqrt(var + eps), in place over all var rows
                nc.scalar.activation(
                    out=res[:, :, 1, :],
                    in_=res[:, :, 1, :],
                    func=mybir.ActivationFunctionType.Sqrt,
                    bias=eps_t[:],
                    scale=1.0,
                )
                if t < NT - 1:
                    # SWDGE (GpSimd) ring: a result-dependent DMA here
                    # would stall the Sync input FIFO mid-stream
                    nc.gpsimd.dma_start(out=out[t * P:(t + 1) * P, :],
                                        in_=res[:])
                else:
                    # input FIFO is drained by now; HWDGE is ~1.5us
                    # faster end-to-end than SWDGE for the final store
                    nc.sync.dma_start(out=out[t * P:(t + 1) * P, :],
                                      in_=res[:])
    nc.compile()
    return nc


def _ensure_ntff_shim():
    """bass_utils imports antenv.axon_hooks when tracing is requested
    (trace=True or BASS_TRACE=1); some images lack that module. Provide a
    functional shim backed by trn_boot's ctypes NTFF hook when possible,
    else a no-op that degrades tracing gracefully."""
    import sys
    import types
    try:
        import antenv.axon_hooks  # noqa: F401
        return
    except ImportError:
        pass
    try:
        import antenv
    except ImportError:
        return
    mod = types.ModuleType("antenv.axon_hooks")
    mod._hook = None
    mod.set_axon_ntff_profile_hook = lambda h: setattr(mod, "_hook", h)
    mod.get_axon_ntff_profile_hook = lambda: mod._hook
    try:
        from trn_agent_boot.trn_boot import _ntff_profile_via_ctypes
        mod.set_axon_ntff_profile_hook(
            _ntff_profile_via_ctypes("/opt/axon/libaxon_pjrt.so"))
    except Exception:
        pass
    sys.modules["antenv.axon_hooks"] = mod
    antenv.axon_hooks = mod


def kernel(x: np.ndarray) -> np.ndarray:
    global LAST_RESULTS
    _ensure_ntff_shim()
    from concourse.bass_utils import run_bass_kernel_spmd

    if "nc" not in _CACHE:
        _CACHE["nc"] = _build()
    nc = _CACHE["nc"]

    x = np.ascontiguousarray(np.asarray(x, dtype=np.float32))
    in_maps = [
        {"x": x[i * B_SH:(i + 1) * B_SH].reshape(BC, HW)}
        for i in range(N_CORES)
    ]
    trace = bool(int(os.environ.get("KERNEL_TRACE", "0")))
    res = run_bass_kernel_spmd(nc, in_maps, core_ids=list(range(N_CORES)),
                               trace=trace)
    LAST_RESULTS = res
    out = np.concatenate(
        [res.results[i]["out"].reshape(B_SH, C, 2 * OUT_H2, OUT_W)
         for i in range(N_CORES)],
        axis=0,
    )
    return out


# revision 19
# speedup vs baseline: 1.0131x; 1.0131x over previous
"""AdaptiveStdPool2d kernel for Trainium2 (8 NeuronCores, data-parallel).

Input  x: [32, 64, 512, 80] f32
Output:   [32, 64, 8, 10] f32  (mean/std interleaved along height)

Math: per (b, c), split H=512 into 4 windows of 128 and W=80 into 10
windows of 8; out[b,c,2*oh,ow] = mean of 128x8 window, out[b,c,2*oh+1,ow]
= sqrt(biased_var + 1e-14).

Strategy (per core, shard = 4 batches -> 256 (b,c) rows):
- partition dim = (b,c) row (2 tiles of 128), free dim = flattened (h,w).
- per (bc-tile, oh): one contiguous slab DMA [128, 10240] on the Sync
  HWDGE ring (FIFO -> continuous HBM stream at ~424 GB/s).  The LAST
  slab is DMA'd as rows 0..95 + 96..127 so only a quarter of its work
  trails the final bytes (shorter drain tail).
- output DMAs stay OFF the hot Sync FIFO: tile 0's result goes out on
  the GpSimd SWDGE ring (a result-dependent wait on the Sync ring would
  head-of-line-block input descgen mid-stream; the Scalar HWDGE ring
  measurably perturbs SDMA engine 15 - do not use it); the final
  result uses the Sync ring, which is drained by then and ~1.5us
  faster end-to-end than SWDGE.
- window stats split across engines on every slab: 7 of 10 windows on
  DVE (2x BNStats 512-elem groups + BNStatsAggregate straight into the
  interleaved output layout), 3 on ACT (Square/Copy with scale +
  accum_out giving E[x^2] and mean directly), 8/2 on the last slab.
  Per-slab work on each engine depends ONLY on that slab's DMA - no
  per-slab cross-engine sync, so one straggling DMA engine can't
  cascade.
- finishing (var = E[x^2]-mean^2 for ACT windows, sqrt(var+eps) for
  all) is batched once per bc-tile: a few tiny DVE ops + 1 ACT sqrt.
"""

import os
import numpy as np

B, C, H, W = 32, 64, 512, 80
OUT_H2, OUT_W = 4, 10
WH, WW = H // OUT_H2, W // OUT_W  # 128, 8
EPS = 1e-14
NWIN = WH * WW                   # 1024 elements per window

N_CORES = 8
B_SH = B // N_CORES          # 4 batches per core
BC = B_SH * C                # 256 rows per core
HW = H * W                   # 40960
SLAB = WH * W                # 10240 elements per (oh) slab
HALF = SLAB // 2             # 5120 (rows 0..63 of a slab)
OUT_FREE = 2 * OUT_H2 * OUT_W  # 80 output elements per (b,c)

_CACHE = {}
LAST_RESULTS = None


def _build():
    import concourse.bacc as bacc
    import concourse.tile as tile
    from concourse import mybir

    nc = bacc.Bacc("TRN2", target_bir_lowering=False, debug=False)

    x = nc.dram_tensor("x", [BC, HW], mybir.dt.float32, kind="ExternalInput")
    out = nc.dram_tensor("out", [BC, OUT_FREE], mybir.dt.float32,
                         kind="ExternalOutput")

    P = 128
    NT = BC // P  # 2 bc-tiles
    INV_N = 1.0 / NWIN
    INV_SQRTN = 1.0 / 32.0   # 1/sqrt(1024): Square(x/32) accums to E[x^2]

    def bn_stats_raw(in_ap, out_ap):
        # Raw InstBNStats: HW computes one 6-stat group over the whole
        # (multi-dim) input AP; bass's wrapper would reject this shape.
        nc.vector.add_instruction(
            mybir.InstBNStats(
                name=nc.get_next_instruction_name(),
                ins=[nc.vector.lower_ap(in_ap)],
                outs=[nc.vector.lower_ap(out_ap)],
            )
        )

    with tile.TileContext(nc) as tc:
        with (
            tc.tile_pool(name="slabs", bufs=4) as slabs,
            tc.tile_pool(name="scratch", bufs=2) as scratch_pool,
            tc.tile_pool(name="small", bufs=4) as small,
            tc.tile_pool(name="res", bufs=2) as res_pool,
            tc.tile_pool(name="singles", bufs=1) as singles,
        ):
            eps_t = singles.tile([P, 1], mybir.dt.float32)
            nc.vector.memset(eps_t, EPS)

            # Chain each input DMA on the previous one's completion sem.
            # The ~1.7us issue latency per link caps this core's HBM
            # demand at ~380 GB/s (vs 424 free-running), close to its
            # fair half of the shared 716 GB/s stack - the paired
            # NeuronCore then isn't starved, which cuts the worst-core
            # (contention-loser) time that dominates max-over-cores.
            prev_dma = [None]

            def chained_dma(out_ap, in_ap):
                d = nc.sync.dma_start(out=out_ap, in_=in_ap)
                if prev_dma[0] is not None:
                    tile.add_dep_helper(d.ins, prev_dma[0].ins, sync=True,
                                        reason="hbm demand throttle")
                prev_dma[0] = d
                return d

            for t in range(NT):
                res = res_pool.tile([P, OUT_H2, 2, OUT_W], mybir.dt.float32)
                # E[x^2] for the ACT windows, kept per tile until the
                # batched finishing pass.
                ex2 = small.tile([P, OUT_H2, OUT_W], mybir.dt.float32)
                for oh in range(OUT_H2):
                    last = (t == NT - 1) and (oh == OUT_H2 - 1)
                    n_act = 2 if last else 3
                    n_dve = OUT_W - n_act
                    slab = slabs.tile([P, SLAB], mybir.dt.float32)
                    c0 = oh * SLAB
                    slab_v = slab.rearrange("p (r ow w) -> p ow r w",
                                            ow=OUT_W, w=WW)
                    stats = small.tile([P, OUT_W, 2, 6], mybir.dt.float32)
                    if not last:
                        chained_dma(slab[:],
                                    x[t * P:(t + 1) * P, c0:c0 + SLAB])
                        # DVE windows: 2x 512-elem column-split groups
                        for ow in range(n_dve):
                            for k in range(2):
                                bn_stats_raw(
                                    slab_v[:, ow, :, k * 4:(k + 1) * 4],
                                    stats[:, ow, k, :],
                                )
                            nc.vector.bn_aggr(out=res[:, oh, :, ow],
                                              in_=stats[:, ow, :, :])
                        # ACT windows: E[x^2] and mean via scaled accum
                        for ow in range(n_dve, OUT_W):
                            sq_scr = scratch_pool.tile([P, WH, WW],
                                                       mybir.dt.float32)
                            nc.scalar.activation(
                                out=sq_scr[:],
                                in_=slab_v[:, ow],
                                func=mybir.ActivationFunctionType.Square,
                                scale=INV_SQRTN,
                                accum_out=ex2[:, oh, ow:ow + 1],
                            )
                            cp_scr = scratch_pool.tile([P, WH, WW],
                                                       mybir.dt.float32)
                            nc.scalar.activation(
                                out=cp_scr[:],
                                in_=slab_v[:, ow],
                                func=mybir.ActivationFunctionType.Copy,
                                scale=INV_N,
                                accum_out=res[:, oh, 0, ow:ow + 1],
                            )
                    else:
                        # Last slab: DMA as rows 0..95 + rows 96..127 so
                        # only a quarter of the work trails the final
                        # chunk.  DVE: 3 row-group BNStats per window
                        # (<=512 elems each); ACT: partial accums per
                        # chunk, summed on DVE.
                        RA = WH * 3 // 4  # 96
                        CA = RA * W       # 7680
                        chained_dma(slab[:, :CA],
                                    x[t * P:(t + 1) * P, c0:c0 + CA])
                        chained_dma(slab[:, CA:],
                                    x[t * P:(t + 1) * P, c0 + CA:c0 + SLAB])
                        stats3 = small.tile([P, OUT_W, 3, 6],
                                            mybir.dt.float32)
                        # chunk A work (overlaps chunk-B DMA)
                        for ow in range(n_dve):
                            bn_stats_raw(slab_v[:, ow, 0:WH // 2, :],
                                         stats3[:, ow, 0, :])
                            bn_stats_raw(slab_v[:, ow, WH // 2:RA, :],
                                         stats3[:, ow, 1, :])
                        sm_p = small.tile([P, n_act, 2], mybir.dt.float32)
                        ex_p = small.tile([P, n_act, 2], mybir.dt.float32)
                        for j, ow in enumerate(range(n_dve, OUT_W)):
                            sq_scr = scratch_pool.tile([P, WH, WW],
                                                       mybir.dt.float32)
                            nc.scalar.activation(
                                out=sq_scr[:, :RA],
                                in_=slab_v[:, ow, :RA],
                                func=mybir.ActivationFunctionType.Square,
                                scale=INV_SQRTN,
                                accum_out=ex_p[:, j, 0:1],
                            )
                            cp_scr = scratch_pool.tile([P, WH, WW],
                                                       mybir.dt.float32)
                            nc.scalar.activation(
                                out=cp_scr[:, :RA],
                                in_=slab_v[:, ow, :RA],
                                func=mybir.ActivationFunctionType.Copy,
                                scale=INV_N,
                                accum_out=sm_p[:, j, 0:1],
                            )
                        # chunk B work (the short tail)
                        for ow in range(n_dve):
                            bn_stats_raw(slab_v[:, ow, RA:, :],
                                         stats3[:, ow, 2, :])
                            nc.vector.bn_aggr(out=res[:, oh, :, ow],
                                              in_=stats3[:, ow, :, :])
                        for j, ow in enumerate(range(n_dve, OUT_W)):
                            sq_scr = scratch_pool.tile([P, WH, WW],
                                                       mybir.dt.float32)
                            nc.scalar.activation(
                                out=sq_scr[:, RA:],
                                in_=slab_v[:, ow, RA:],
                                func=mybir.ActivationFunctionType.Square,
                                scale=INV_SQRTN,
                                accum_out=ex_p[:, j, 1:2],
                            )
                            cp_scr = scratch_pool.tile([P, WH, WW],
                                                       mybir.dt.float32)
                            nc.scalar.activation(
                                out=cp_scr[:, RA:],
                                in_=slab_v[:, ow, RA:],
                                func=mybir.ActivationFunctionType.Copy,
                                scale=INV_N,
                                accum_out=sm_p[:, j, 1:2],
                            )
                        # combine chunk partials
                        nc.vector.tensor_add(res[:, oh, 0, n_dve:],
                                             sm_p[:, :, 0], sm_p[:, :, 1])
                        nc.vector.tensor_add(ex2[:, oh, n_dve:],
                                             ex_p[:, :, 0], ex_p[:, :, 1])
                # Batched finishing for the whole tile.
                # var for ACT windows: E[x^2] - mean^2.  The last slab
                # of the last tile ran only ow 8:10 on ACT (ow 7 came
                # from bn_aggr), so its range is ragged.
                m2 = small.tile([P, OUT_H2, OUT_W], mybir.dt.float32)
                if t == NT - 1:
                    regions = [(slice(0, OUT_H2 - 1), slice(7, OUT_W)),
                               (OUT_H2 - 1, slice(8, OUT_W))]
                else:
                    regions = [(slice(0, OUT_H2), slice(7, OUT_W))]
                for rh, rw in regions:
                    nc.vector.tensor_mul(m2[:, rh, rw],
                                         res[:, rh, 0, rw],
                                         res[:, rh, 0, rw])
                    nc.vector.scalar_tensor_tensor(
                        out=res[:, rh, 1, rw],
                        in0=ex2[:, rh, rw],
                        scalar=1.0,
                        in1=m2[:, rh, rw],
                        op0=mybir.AluOpType.mult,
                        op1=mybir.AluOpType.subtract,
                    )
                # std = sqrt(var + eps), in place over all var rows
                nc.scalar.activation(
                    out=res[:, :, 1, :],
                    in_=res[:, :, 1, :],
                    func=mybir.ActivationFunctionType.Sqrt,
                    bias=eps_t[:],
                    scale=1.0,
                )
                if t < NT - 1:
                    # SWDGE (GpSimd) ring: a result-dependent DMA here
                    # would stall the Sync input FIFO mid-stream
                    nc.gpsimd.dma_start(out=out[t * P:(t + 1) * P, :],
                                        in_=res[:])
                else:
                    # input FIFO is drained by now; HWDGE is ~1.5us
                    # faster end-to-end than SWDGE for the final store
                    nc.sync.dma_start(out=out[t * P:(t + 1) * P, :],
                                      in_=res[:])
    nc.compile()
    return nc


def _ensure_ntff_shim():
    """bass_utils imports antenv.axon_hooks when tracing is requested
    (trace=True or BASS_TRACE=1); some images lack that module. Provide a
    functional shim backed by trn_boot's ctypes NTFF hook when possible,
    else a no-op that degrades tracing gracefully."""
    import sys
    import types
    try:
        import antenv.axon_hooks  # noqa: F401
        return
    except ImportError:
        pass
    try:
        import antenv
    except ImportError:
        return
    mod = types.ModuleType("antenv.axon_hooks")
    mod._hook = None
    mod.set_axon_ntff_profile_hook = lambda h: setattr(mod, "_hook", h)
    mod.get_axon_ntff_profile_hook = lambda: mod._hook
    try:
        from trn_agent_boot.trn_boot import _ntff_profile_via_ctypes
        mod.set_axon_ntff_profile_hook(
            _ntff_profile_via_ctypes("/opt/axon/libaxon_pjrt.so"))
    except Exception:
        pass
    sys.modules["antenv.axon_hooks"] = mod
    antenv.axon_hooks = mod


def kernel(x: np.ndarray) -> np.ndarray:
    global LAST_RESULTS
    _ensure_ntff_shim()
    from concourse.bass_utils import run_bass_kernel_spmd

    if "nc" not in _CACHE:
        _CACHE["nc"] = _build()
    nc = _CACHE["nc"]

    x = np.ascontiguousarray(np.asarray(x, dtype=np.float32))
    in_maps = [
        {"x": x[i * B_SH:(i + 1) * B_SH].reshape(BC, HW)}
        for i in range(N_CORES)
    ]
    trace = bool(int(os.environ.get("KERNEL_TRACE", "0")))
    res = run_bass_kernel_spmd(nc, in_maps, core_ids=list(range(N_CORES)),
                               trace=trace)
    LAST_RESULTS = res
    out = np.concatenate(
        [res.results[i]["out"].reshape(B_SH, C, 2 * OUT_H2, OUT_W)
         for i in range(N_CORES)],
        axis=0,
    )
    return out
